# revision 23
# baseline (speedup 1.0000x reference)
"""Causal self-attention (B=4, T=2048, C=1024, H=16, D=64) on 8 TRN2 cores.

Sharding: 2 cores per batch element; core c -> batch c//2, heads
(c%2)*8 .. +8.  Each core computes the partial projection output for its
heads' columns of w_proj; the host sums the two partials per batch.  No
collectives.

Engine balance vs the original version (which was ACT-bound in stage B
and spent PE time on x^T transposes in stage A):
  stage A: x / wqk / wv arrive as bf16 from the host; x^T comes from
           DMA-transpose (DRAM -> SBUF xbar path, issued on the idle ACT
           hwdge queue) instead of PE transposes; weights DMA straight
           into their operand tiles (no staging copies; wproj is f32r =
           f32 bits, no rounding op needed on the DMA path).  q/k GEMMs
           run at the full N=512 moving width; all PSUM->SBUF copies on
           DVE.
  stage B: per (512-wide q-strip, head): S^T = k @ q^T with T_k on the
           PSUM partition axis; exp on ACT straight out of PSUM
           (scale 1/sqrt(D), no max-shift: logits are ~N(0,1)); causal
           0/1 mask multiply (DVE) only on the [128,128] diagonal
           sub-blocks - the fully-masked region left of a diagonal block
           is skipped by restricting the PV moving range (PV_Q0) instead
           of masking; out^T[65, q] += [V|1]^T @ P^T per k-chunk (row 64
           is the softmax denominator l).  Normalize r = 1/l as
           exp(-ln(l)) on ACT (same activation table as the bulk Exp -
           see _patch_act_tables), K=1 matmul broadcasts r to partitions
           0-63, DVE multiply writes the projection lhsT strip (odd
           heads bounce via an SBUF->SBUF DMA to reach partitions
           64-127).  The whole normalize chain is emitted one head LATE
           so its pb matmul never stalls the PE queue waiting on ACT.

HW-vs-sim divergences found while developing (kept out of the kernel):
gpsimd.partition_broadcast with an out-partition offset writes wrong
data on HW; the DVE reciprocal_approx_fast -> f32r-cast -> matmul-rhs
chain returns garbage on HW; narrowed strided-AP exp produced NaN on HW.
All pass CoreSim - only op patterns proven on hardware are used here.
"""

import numpy as np
import ml_dtypes

import concourse.mybir as mybir
import concourse.tile as tile
from concourse import bacc
from concourse.bass import ts, ds
from concourse.bass_utils import run_bass_kernel_spmd

B, T, C, H, D = 4, 2048, 1024, 16, 64
HPC = H // 2          # heads per core = 8
N_CORES = 8
P = 128
f32 = mybir.dt.float32
f32r = mybir.dt.float32r
bf16 = mybir.dt.bfloat16

KO = C // P           # 8 contraction subtiles over C
NQ = T // 512         # 4 q-strips
VW = D + 1            # 65: V plus the ones column
NPROJ = HPC * D // P  # 4 contraction subtiles for the projection
NT = T // P           # 16 t-tiles

# S / PV moving-dim restriction for the 4 diagonal k-chunks of a strip
# (jd = kc - 4*qc in 0..3).  S must cover every element the pair-exp call
# reads, so j3 widens to q0=256; PV reads only defined pt so it tightens.
S_Q0 = (0, 0, 256, 256)
PV_Q0 = (0, 128, 256, 384)


def _patch_act_tables():
    """Steer Exp and Ln to the one activation-table set that contains both
    (natural_log_exp_and_others).  By default the table-load inserter picks
    per-function sets, which makes the per-head Ln thrash the ACT table
    against the bulk Exp ops: 64 ACT_TABLE_LOADs x 1.28us measured.  Set ids
    are positional, so entries are neutered in place, never reordered."""
    import functools
    import concourse.hw_specs as hw_specs
    if getattr(hw_specs, "_act_tables_patched", False):
        return
    orig = hw_specs.get_activation_tables

    @functools.cache
    def patched(arch):
        tabs = {k: set(v) for k, v in orig(arch).items()}
        keep = "natural_log_exp_and_others"
        if keep in tabs:
            for name, fns in tabs.items():
                if name != keep:
                    fns.discard(mybir.ActivationFunctionType.Exp)
                    fns.discard(mybir.ActivationFunctionType.Ln)
        return tabs

    hw_specs.get_activation_tables = patched
    bacc.get_activation_tables = patched
    hw_specs._act_tables_patched = True


def _build_module():
    _patch_act_tables()
    nc = bacc.Bacc()
    xb = nc.dram_tensor("xb", [T, C], bf16, kind="ExternalInput")
    wqk = nc.dram_tensor("wqk", [C, HPC * P], bf16, kind="ExternalInput")
    wv = nc.dram_tensor("wv", [C, HPC * D], bf16, kind="ExternalInput")
    wproj = nc.dram_tensor("wproj", [HPC * D, C], f32r, kind="ExternalInput")
    outp = nc.dram_tensor("outp", [T, C], f32, kind="ExternalOutput")

    with tile.TileContext(nc) as tc:
        with tc.tile_pool(name="persist", bufs=1) as persist:
            qT = persist.tile([P, HPC // 2, T], bf16, tag="qT")          # 2 MB
            kT = persist.tile([P, HPC // 2, T], bf16, tag="kT")          # 2 MB
            v_sb = persist.tile([P, NT, HPC, VW], bf16, tag="v_sb")  # 2.2 MB
            gmask0 = persist.tile([P, P], bf16, tag="gmask0")
            ones1 = persist.tile([P, 1], f32, tag="ones1")
            onesb = persist.tile([VW, D], f32r, tag="onesb")  # row 64
            wproj_r = persist.tile([P, NPROJ, C], f32r, tag="wproj_r")   # 2 MB
            wqk_r = persist.tile([P, KO, HPC * P], bf16, tag="wqk_r")    # 2 MB
            wv_r = persist.tile([P, KO, HPC * D], bf16, tag="wv_r")      # 1 MB

            # weights arrive pre-cast to bf16 from the host: DMA
            # straight into the matmul operand tiles, no staging, no
            # casts.  wproj (f32r == f32 bits) DMA'd direct, last.
            for hf in range(2):
                nc.gpsimd.dma_start(
                    wqk_r[:, ds(hf * (KO // 2), KO // 2), :],
                    wqk.rearrange("(ko p) n -> p ko n", p=P)
                    [:, ds(hf * (KO // 2), KO // 2), :])
            nc.gpsimd.dma_start(
                wv_r[:], wv.rearrange("(ko p) n -> p ko n", p=P))
            for ko in range(NPROJ):
                nc.gpsimd.dma_start(wproj_r[:, ko, :], wproj[ts(ko, P), :])

            nc.gpsimd.memset(ones1[:], 1.0)
            # causal keep-mask for a diagonal [128,128] sub-block:
            # gmask0[p, q] = 1 iff p <= q
            nc.gpsimd.memset(gmask0[:], 1.0)
            nc.gpsimd.affine_select(
                out=gmask0[:], in_=gmask0[:],
                compare_op=mybir.AluOpType.is_ge, fill=0.0,
                base=0, pattern=[[1, P]], channel_multiplier=-1)
            # ones row for the K=1 r-broadcast matmul
            nc.vector.tensor_copy(
                onesb[D:VW, :], ones1[D:VW, 0:1].broadcast_to([1, D]))
            # ones column of [V | 1]
            nc.vector.tensor_copy(
                v_sb[:, :, :, D:VW],
                ones1[:, None, :].broadcast_to([P, NT, HPC, 1]))

            # ---------------- stage A: qkv projection ----------------
            with tc.tile_pool(name="xT_p", bufs=2) as xT_p, \
                 tc.tile_pool(name="ps_qk", bufs=2, space="PSUM") as ps_qk, \
                 tc.tile_pool(name="ps_v", bufs=2, space="PSUM") as ps_v:


                for ch in range(NQ):
                    xT = xT_p.tile([P, KO, 512], bf16, tag="xT")
                    nc.scalar.dma_start_transpose(xT[:], xb[ts(ch, 512), :])
                    # q^T (pairs 0-3) then k^T (pairs 4-7) for this chunk
                    for g in range(HPC):
                        pqk = ps_qk.tile([P, 512], f32, tag="pqk")
                        for ko in range(KO):
                            nc.tensor.matmul(
                                pqk[:], wqk_r[:, ko, ts(g, P)], xT[:, ko, :],
                                start=(ko == 0), stop=(ko == KO - 1))
                        dst = qT if g < HPC // 2 else kT
                        nc.vector.tensor_copy(
                            dst[:, g % (HPC // 2), ts(ch, 512)], pqk[:])
                    # V rows for the four t-tiles of this chunk
                    for sub in range(4):
                        pv = ps_v.tile([P, HPC * D], f32, tag="pv")
                        for ko in range(KO):
                            nc.tensor.matmul(
                                pv[:], xT[:, ko, ts(sub, P)], wv_r[:, ko, :],
                                start=(ko == 0), stop=(ko == KO - 1))
                        tt = ch * 4 + sub
                        nc.vector.tensor_copy(
                            v_sb[:, tt, :, 0:D],
                            pv[:, None, :].rearrange("p one (h d) -> p (one h) d", d=D))

            # ------------- stage B: attention + projection -------------
            with tc.tile_pool(name="pt_p", bufs=6) as pt_p, \
                 tc.tile_pool(name="strip_p", bufs=2) as strip_p, \
                 tc.tile_pool(name="small", bufs=2) as small, \
                 tc.tile_pool(name="out_p", bufs=2) as out_p, \
                 tc.tile_pool(name="ps_s", bufs=2, space="PSUM") as ps_s, \
                 tc.tile_pool(name="ps_o", bufs=2, space="PSUM") as ps_o, \
                 tc.tile_pool(name="ps_p", bufs=1, space="PSUM") as ps_p, \
                 tc.tile_pool(name="ps_b", bufs=1, space="PSUM") as ps_b:

                pending_proj = []
                for qc in (3, 2, 1, 0):
                    # heaviest strip first: keeps the PE dense right after
                    # stage A and leaves the lightest strip for the tail
                    strip = strip_p.tile([P, NPROJ, 512], f32r, tag="strip")

                    def emit_head(h, po):
                        odd = h % 2
                        off = odd * D
                        g2 = h // 2
                        nk = 4 * (qc + 1)          # causal k-chunks

                        def emit_s_exp(kg):
                            # S^T block-group matmuls + exp (+causal 0/1 mask
                            # multiply on the diagonal [128,128] sub-blocks;
                            # columns left of a diagonal block are skipped by
                            # the PV read restriction instead)
                            pss = ps_s.tile([P, 2, 512], f32, tag="pss")
                            pt = pt_p.tile([P, 2, 512], bf16, tag="pt")
                            for j2 in range(2):
                                kc = kg * 2 + j2
                                jd = kc - 4 * qc
                                q0 = S_Q0[jd] if jd >= 0 else 0
                                nc.tensor.matmul(
                                    pss[:, j2, q0:512],
                                    kT[off:off + D, g2, ts(kc, P)],
                                    qT[off:off + D, g2,
                                       ds(qc * 512 + q0, 512 - q0)],
                                    start=True, stop=True)
                            nc.scalar.activation(
                                pt[:], pss[:],
                                mybir.ActivationFunctionType.Exp,
                                scale=float(1.0 / np.sqrt(D)))
                            if kg >= 2 * qc:          # diagonal pair
                                for j2 in range(2):
                                    jd = kg * 2 + j2 - 4 * qc
                                    blk = pt[:, j2, ds(jd * P, P)]
                                    nc.vector.tensor_tensor(
                                        blk, blk, gmask0[:],
                                        mybir.AluOpType.mult)
                            return pt

                        def emit_pv(kg, pt):
                            for j2 in range(2):
                                kc = kg * 2 + j2
                                jd = kc - 4 * qc
                                q0 = PV_Q0[jd] if jd >= 0 else 0
                                nc.tensor.matmul(
                                    po[:, q0:512], v_sb[:, kc, h, :],
                                    pt[:, j2, q0:512],
                                    start=(kc == 0), stop=(kc == nk - 1),
                                    skip_group_check=True)

                        # software-pipelined: the next group's S matmuls sit
                        # ahead of this group's PV in PE program order, so the
                        # PE never stalls on the ACT exp latency.
                        prev = None
                        for kg in range(nk // 2):
                            pt = emit_s_exp(kg)
                            if prev is not None:
                                emit_pv(kg - 1, prev)
                            prev = pt
                        emit_pv(nk // 2 - 1, prev)

                    def emit_norm(h, po):
                        # normalize head h: r = 1/l as exp(-ln(l)) on ACT, K=1
                        # matmul broadcasts r into partitions 0-63, DVE
                        # multiply writes the projection lhsT strip (odd heads
                        # bounce via an SBUF->SBUF DMA to reach partitions
                        # 64-127).  Emitted one head LATE so the pb matmul
                        # never stalls the PE queue on the ACT Ln/Exp chain.
                        odd = h % 2
                        g2 = h // 2
                        l64 = small.tile([VW, 512], f32, tag="l64")
                        nc.scalar.activation(l64[D:VW, :], po[D:VW, :],
                                             mybir.ActivationFunctionType.Ln)
                        r64 = small.tile([VW, 512], f32r, tag="r64")
                        nc.scalar.activation(r64[D:VW, :], l64[D:VW, :],
                                             mybir.ActivationFunctionType.Exp,
                                             scale=-1.0)
                        pb = ps_b.tile([D, 512], f32, tag="pb")
                        nc.tensor.matmul(pb[:], onesb[D:VW, :],
                                         r64[D:VW, :], start=True, stop=True)
                        att = small.tile([D, 512], f32, tag="att")
                        nc.vector.tensor_copy(att[:], po[0:D, :])
                        if not odd:
                            nc.vector.tensor_tensor(
                                strip[0:D, g2, :], att[:], pb[:],
                                mybir.AluOpType.mult)
                        else:
                            tmp = small.tile([D, 512], f32r, tag="tmp")
                            nc.vector.tensor_tensor(
                                tmp[:], att[:], pb[:],
                                mybir.AluOpType.mult)
                            nc.sync.dma_start(strip[D:P, g2, :], tmp[:])

                    po_prev = None
                    for h in range(HPC):
                        po = ps_o.tile([VW, 512], f32, tag="po")
                        emit_head(h, po)
                        if pending_proj:
                            # one projection group of the previous strip per
                            # head: spaces the single-buffered pp bank's
                            # matmul->copy chain out so the PE never waits
                            pending_proj.pop(0)()
                        if po_prev is not None:
                            emit_norm(h - 1, po_prev)
                        po_prev = po
                    emit_norm(HPC - 1, po_prev)
                    while pending_proj:
                        pending_proj.pop(0)()

                    def make_proj(s_tile, qc_, tsub, nch):
                        def emit():
                            pp = ps_p.tile([P, 512], f32, tag="pp")
                            for ko in range(NPROJ):
                                nc.tensor.matmul(
                                    pp[:], s_tile[:, ko, ts(tsub, P)],
                                    wproj_r[:, ko, ts(nch, 512)],
                                    start=(ko == 0), stop=(ko == NPROJ - 1))
                            osb = out_p.tile([P, 512], f32, tag="osb")
                            nc.vector.tensor_copy(osb[:], pp[:])
                            nc.sync.dma_start(
                                outp[ds(qc_ * 512 + tsub * P, P),
                                     ts(nch, 512)], osb[:])
                        return emit

                    for tsub in range(4):
                        for nch in range(2):
                            pending_proj.append(make_proj(strip, qc, tsub, nch))
                # flush the last strip's projection
                while pending_proj:
                    pending_proj.pop(0)()

    nc.finalize()
    return nc


_NC_CACHE = None


def _get_module():
    global _NC_CACHE
    if _NC_CACHE is None:
        _NC_CACHE = _build_module()
    return _NC_CACHE


def _core_inputs(x, w_qkv, w_proj, c):
    """Slice + relayout the full inputs for core c."""
    b, hg = c // 2, c % 2
    h0 = hg * HPC
    # wqk: cols 0-511 = q for the 8 heads (pair layout: pair g2 holds head
    # h0+2*g2 in cols [g2*128, +64) and head h0+2*g2+1 in [g2*128+64, +64)),
    # cols 512-1023 = k in the same layout.
    wqk_c = np.empty((C, HPC * P), dtype=np.float32)
    for g2 in range(HPC // 2):
        for par in range(2):
            h = h0 + 2 * g2 + par
            col = g2 * P + par * D
            wqk_c[:, col:col + D] = w_qkv[:, h * D:(h + 1) * D]
            wqk_c[:, 512 + col:512 + col + D] = \
                w_qkv[:, C + h * D:C + (h + 1) * D]
    wv_c = w_qkv[:, 2 * C + h0 * D:2 * C + (h0 + HPC) * D]
    # wproj rows must match the strip layout: row ko*128 + p corresponds to
    # head h0 + 2*ko + p//64, dim p%64.
    wproj_c = np.empty((HPC * D, C), dtype=np.float32)
    for ko in range(NPROJ):
        for par in range(2):
            h = h0 + 2 * ko + par
            row = ko * P + par * D
            wproj_c[row:row + D, :] = w_proj[h * D:(h + 1) * D, :]
    return {
        "xb": np.ascontiguousarray(x[b]).astype(ml_dtypes.bfloat16),
        "wqk": wqk_c.astype(ml_dtypes.bfloat16),
        "wv": np.ascontiguousarray(wv_c).astype(ml_dtypes.bfloat16),
        "wproj": wproj_c,
    }


def kernel(x: np.ndarray, w_qkv: np.ndarray, w_proj: np.ndarray) -> np.ndarray:
    x = np.ascontiguousarray(np.asarray(x, dtype=np.float32))
    w_qkv = np.ascontiguousarray(np.asarray(w_qkv, dtype=np.float32))
    w_proj = np.ascontiguousarray(np.asarray(w_proj, dtype=np.float32))

    nc = _get_module()
    in_maps = [_core_inputs(x, w_qkv, w_proj, c) for c in range(N_CORES)]
    res = run_bass_kernel_spmd(nc, in_maps, core_ids=list(range(N_CORES)))
    out = np.empty((B, T, C), dtype=np.float32)
    for b in range(B):
        out[b] = res.results[2 * b]["outp"] + res.results[2 * b + 1]["outp"]
    return out
